# revision 55
# baseline (speedup 1.0000x reference)
"""ColorUnpool (gather + segment-max + relu) as an 8-core Trainium2 Bass kernel.

Reference semantics:
    out = zeros([200000, 256]);  out[center_idx] = feat            # centers
    seg = segment_max(feat[edge_src], edge_dst)                    # edges
    out[r] = max(seg[r], 0) for rows r with >= 1 incoming edge

edge_dst only hits rows [50000, 200000) and center_idx only [0, 50000), so
the two regions are disjoint.  The center region is a pure host-side copy of
the input (no compute); the device computes the edge region only.

Device strategy (per core, dst rows split 8 ways -> 18750 rows/core):
  * The feat table is compacted per core to its ~31.6k distinct src rows
    (< 32768) so gather indices fit in int16, and the gather runs as
    1024-index `dma_gather` instructions (the HW cap) round-robined over
    all 4 SWDGE queues -- descriptor generation for different queues runs
    concurrently on the Q7 cores, which quarters the ~7.7ns/row software
    DGE cost that serialized the old per-column indirect-DMA design.
  * Pair packing: a greedy matching gives ~90% of deg>=2 rows one pair of
    same-row srcs placed in adjacent table rows (2q, 2q+1), so one 1KB
    descriptor (48ns DMA vs 2x33.6ns, one Q7 idx instead of two) fetches
    two edges at once.  Rows are split into block A (paired rows, degree
    desc) and block BC (unpaired deg>=2, then deg<=1 rows, degree desc);
    each block keeps a tight monotone per-tile round structure.
  * Column layout: one pair round over all A tiles (slot q fetches pair q;
    zero pads) whose fused DVE op  acc = max(max(gA, 0), gB)  initializes
    the accumulator relu included; a direct round 0 for BC tiles gathers
    edge 0 straight into the accumulator; then A single rounds and BC
    single rounds (prefix per-tile-max, ZID pads) fold in with DVE maxes
    (BC round 1 fuses the relu, BC tiles with deg<=1 get an
    Activation-engine relu).  Finished tiles are written back as soon as
    their last round completes, overlapping output DMA with gathers.
  * A dummy 16-idx gather triggers the Q7 mlp library IRAM load during
    the preamble; feat is bf16 on device (rel err ~4e-3 << 2e-2 gate);
    the host un-permutes rows and upcasts to f32.
"""

import sys
import types

import numpy as np
import ml_dtypes

sys.path.insert(0, "/opt/trn_rl_repo")

N_NODES = 200000
N_CENTERS = 50000
FEAT = 256
NCORES = 8
P = 128

R_EDGE = N_NODES - N_CENTERS          # 150000 edge-target rows
RC = R_EDGE // NCORES                 # 18750 edge rows per core
TILES = (RC + P - 1) // P             # 147 tiles of 128 rows
NPOS = TILES * P                      # 18816 padded row slots
TBL = 32768                           # per-core compact feat table rows
ZID = TBL - 1                         # zero single row (table zero-padded)
NPAIR = TBL // 2                      # pair view [16384, 512]
G = 8                                 # gather chunk width (cols); HW caps a
                                      # single dma_gather at 1024 indices
WMIN = 8                              # min writeback width (tiles)


def _install_profile_hook():
    """Provide antenv.axon_hooks (missing on this image) so that
    run_bass_kernel_spmd(trace=True) can profile via the axon .so."""
    try:
        import antenv
        if "antenv.axon_hooks" in sys.modules:
            return
        from trn_agent_boot.trn_boot import _ntff_profile_via_ctypes
        mod = types.ModuleType("antenv.axon_hooks")
        hook = _ntff_profile_via_ctypes("/opt/axon/libaxon_pjrt.so")
        mod.get_axon_ntff_profile_hook = lambda: hook
        mod.set_axon_ntff_profile_hook = lambda h: None
        sys.modules["antenv.axon_hooks"] = mod
        antenv.axon_hooks = mod
    except Exception:
        pass


def _prep_core(ld, ss):
    """CSR + greedy one-pair-per-row matching for one core."""
    deg = np.bincount(ld, minlength=RC)
    eo = np.argsort(ld, kind="stable")
    ss_sorted = ss[eo]
    starts = np.concatenate([[0], np.cumsum(deg)[:-1]])
    uniq, inv = np.unique(ss_sorted, return_inverse=True)
    U = len(uniq)
    assert U + 2 < TBL, f"{U} distinct srcs > int16 budget"

    bydeg = np.argsort(-deg, kind="stable")
    free = np.ones(U, bool)
    row_pair = [None] * RC                # (src_a, src_b) or None
    row_srcs = [None] * RC                # distinct src ids
    for r in bydeg:
        d = int(deg[r])
        if d == 0:
            row_srcs[r] = np.empty(0, np.int64)
            continue
        srcs = np.unique(inv[starts[r]:starts[r] + d])
        row_srcs[r] = srcs
        if d >= 2:
            cand = [int(s) for s in srcs if free[s]]
            prs = []
            for i in range(0, min(len(cand) - 1, 3), 2):
                a, b = cand[i], cand[i + 1]
                free[a] = False
                free[b] = False
                prs.append((a, b))
            if prs:
                row_pair[r] = prs                        # 1 or 2 pairs
    return dict(deg=deg, uniq=uniq, row_pair=row_pair, row_srcs=row_srcs,
                bydeg=bydeg)


def _build_plan(edge_src, edge_dst, feat):
    """Host preprocessing.  Returns (segs, col_base, C, nA, TA, TBC,
    tables, pair_tables, idx_planes, orders)."""
    edge_src = np.asarray(edge_src, np.int64)
    edge_dst = np.asarray(edge_dst, np.int64)
    local_dst = edge_dst - N_CENTERS
    assert local_dst.min() >= 0 and local_dst.max() < R_EDGE
    core_of = local_dst // RC

    cores = []
    for c in range(NCORES):
        m = core_of == c
        cores.append(_prep_core((local_dst[m] % RC).astype(np.int64),
                                edge_src[m].astype(np.int64)))

    # blocks: A2 (two pairs) and A1 (one pair), each capped at a shared
    # whole-tile count; demoted rows lose pairs and fall through
    nA2 = min(sum(1 for r in range(RC)
                  if pc["row_pair"][r] and len(pc["row_pair"][r]) == 2)
              for pc in cores) // P
    M2 = nA2 * P
    for pc in cores:
        a2 = [r for r in pc["bydeg"]
              if pc["row_pair"][r] and len(pc["row_pair"][r]) == 2]
        for r in a2[M2:]:
            pc["row_pair"][r] = pc["row_pair"][r][:1]    # drop 2nd pair
    nA1 = min(sum(1 for r in range(RC)
                  if pc["row_pair"][r] and len(pc["row_pair"][r]) == 1)
              for pc in cores) // P
    M1 = nA1 * P
    nA = nA2 + nA1
    M = M2 + M1
    assert nA2 >= 1 and nA1 >= 1

    percore = []
    for pc in cores:
        a2_rows = [r for r in pc["bydeg"]
                   if pc["row_pair"][r] and len(pc["row_pair"][r]) == 2]
        a1_rows = [r for r in pc["bydeg"]
                   if pc["row_pair"][r] and len(pc["row_pair"][r]) == 1]
        for r in a1_rows[M1:]:
            pc["row_pair"][r] = None                     # demote to BC
        a1_rows = a1_rows[:M1]
        bc_rows = [r for r in pc["bydeg"] if pc["row_pair"][r] is None]
        # sort A blocks by leftover singles desc -> tight prefix rounds
        a2_rows.sort(key=lambda r: -len(pc["row_srcs"][r]))
        a1_rows.sort(key=lambda r: -len(pc["row_srcs"][r]))
        order = np.array(a2_rows + a1_rows + bc_rows, np.int64)

        # pair id k <-> table rows (2k, 2k+1):
        #   A2 pos q: ids (2q, 2q+1) -> rows 4q..4q+3
        #   A1 pos i: id 2*M2+i -> rows 4*M2+2i, +1
        tbl_row = np.full(len(pc["uniq"]), -1, np.int64)
        for q, r in enumerate(a2_rows):
            (a, b), (c2, d2) = pc["row_pair"][r]
            tbl_row[a] = 4 * q
            tbl_row[b] = 4 * q + 1
            tbl_row[c2] = 4 * q + 2
            tbl_row[d2] = 4 * q + 3
        for i, r in enumerate(a1_rows):
            (a, b), = pc["row_pair"][r]
            tbl_row[a] = 4 * M2 + 2 * i
            tbl_row[b] = 4 * M2 + 2 * i + 1
        rest = np.nonzero(tbl_row < 0)[0]
        assert 4 * M2 + 2 * M1 + len(rest) <= TBL - 1
        tbl_row[rest] = 4 * M2 + 2 * M1 + np.arange(len(rest))

        # per-position singles (ragged): A rows exclude their pairs
        sing = []
        for i, r in enumerate(order):
            srcs = pc["row_srcs"][r]
            if i < M:
                for a, b in pc["row_pair"][r]:
                    srcs = srcs[(srcs != a) & (srcs != b)]
            sing.append(tbl_row[srcs])
        s_len = np.array([len(x) for x in sing] + [0] * (NPOS - RC))
        s_flat = np.concatenate(sing)
        if len(s_flat) == 0:
            s_flat = np.zeros(1, np.int64)
        s_starts = np.concatenate([[0], np.cumsum(s_len)[:-1]])
        SA2 = s_len[:M2].reshape(nA2, P).max(1)
        SA1 = s_len[M2:M].reshape(nA1, P).max(1)
        SBC = s_len[M:].reshape(TILES - nA, P).max(1)    # BC tile max
        percore.append(dict(order=order, tbl_row=tbl_row, uniq=pc["uniq"],
                            s_len=s_len, s_flat=s_flat, s_starts=s_starts,
                            SA2=SA2, SA1=SA1, SBC=SBC))

    def _pwidth(arrs, j):
        # pad-prefix width: 1 + last tile index with value > j (any core);
        # robust to small non-monotonicity (distinct-src count vs degree)
        w = 0
        for a in arrs:
            nz = np.nonzero(a > j)[0]
            if len(nz):
                w = max(w, int(nz[-1]) + 1)
        return w

    TA2 = []                                             # A2 single rounds
    for us in range(max(int(pc["SA2"].max()) for pc in percore)):
        TA2.append(_pwidth([pc["SA2"] for pc in percore], us))
    TA1 = []                                             # A1 single rounds
    for us in range(max(int(pc["SA1"].max()) for pc in percore)):
        TA1.append(_pwidth([pc["SA1"] for pc in percore], us))
    TBC = [TILES - nA]                                   # BC round 0: all
    for js in range(1, max(int(pc["SBC"].max()) for pc in percore)):
        TBC.append(_pwidth([pc["SBC"] for pc in percore], js))

    # trailing BC tiles that are all zero-degree on EVERY core: skip their
    # gather entirely -- the device memzeros + writes them during the idle
    # startup window
    ztiles = TILES
    for pc in percore:
        nz = np.nonzero(pc["SBC"] > 0)[0]
        ztiles = min(ztiles, len(pc["SBC"]) - (int(nz[-1]) + 1 if len(nz)
                                               else 0))
    # sb before sa: the wide BC round-1 writeback lands mid-stream; the
    # narrow deep A2 tail rounds close out the pipeline
    # A2 rows' four paired srcs live at contiguous table rows 4q..4q+3:
    # fetch them with a single 2KB quad descriptor
    segs = [("p4", 0, nA2), ("p1", 0, nA1),
            ("s0", 0, TILES - nA - ztiles)]
    segs += [("sb", js, TBC[js]) for js in range(1, len(TBC)) if TBC[js] > 0]
    segs += [("sa1", us, TA1[us]) for us in range(len(TA1)) if TA1[us] > 0]
    segs += [("sa2", us, TA2[us]) for us in range(len(TA2)) if TA2[us] > 0]
    col_base = np.concatenate([[0], np.cumsum([n for _, _, n in segs])])
    C = int(col_base[-1])

    tables, pair_tables, idx_planes, orders = [], [], [], []
    for pc in percore:
        s_len, s_flat, s_starts = pc["s_len"], pc["s_flat"], pc["s_starts"]
        vals = np.zeros(C * P, np.int64)
        for si, (kind, j, n) in enumerate(segs):
            base = int(col_base[si]) * P
            if kind == "p4":
                vals[base:base + n * P] = np.arange(M2)  # quad q at slot q
                continue
            if kind == "p1":
                vals[base:base + n * P] = 2 * M2 + np.arange(M1)
                continue
            if kind == "sa2":
                qpos = np.arange(n * P)
            elif kind == "sa1":
                qpos = np.arange(M2, M2 + n * P)
            else:                                        # s0 / sb
                qpos = np.arange(M, M + n * P)
            has = s_len[qpos] > j
            v = np.where(has, s_flat[np.minimum(s_starts[qpos] + j,
                                                len(s_flat) - 1)], ZID)
            vals[base:base + n * P] = v
        plane16 = vals.astype(np.int16).reshape(C * 8, 16).T
        idx_planes.append(np.ascontiguousarray(np.tile(plane16, (8, 1))))

        tbl = np.zeros((TBL, FEAT), ml_dtypes.bfloat16)
        tbl[pc["tbl_row"]] = feat[pc["uniq"]].astype(ml_dtypes.bfloat16)
        tables.append(tbl)
        pair_tables.append(tbl.reshape(NPAIR, 2 * FEAT))
        orders.append(pc["order"])
    return (segs, col_base, C, nA2, nA1, ztiles, TA2, TA1, TBC, tables,
            pair_tables, idx_planes, orders)


def _build_bass(segs, col_base, C, nA2, nA1, ztiles, TA2, TA1, TBC):
    import concourse.bacc as bacc
    import concourse.mybir as mybir
    import concourse.tile as tile

    nc = bacc.Bacc("TRN2", target_bir_lowering=False, debug=False,
                   num_devices=NCORES, num_swdge_queues=4)
    t_feat = nc.dram_tensor("feat_tbl", [TBL, FEAT], mybir.dt.bfloat16,
                            kind="ExternalInput")
    t_featp = nc.dram_tensor("feat_tblp", [NPAIR, 2 * FEAT],
                             mybir.dt.bfloat16, kind="ExternalInput")
    t_featq = nc.dram_tensor("feat_tblq", [TBL // 4, 4 * FEAT],
                             mybir.dt.bfloat16, kind="ExternalInput")
    t_idx = nc.dram_tensor("idxs", [P, C * 8], mybir.dt.int16,
                           kind="ExternalInput")
    t_oe = nc.dram_tensor("out_edge", [P, TILES, FEAT], mybir.dt.bfloat16,
                          kind="ExternalOutput")

    mx = mybir.AluOpType.max
    relu = mybir.ActivationFunctionType.Relu
    nA = nA2 + nA1
    TA2_0 = TA2[0] if TA2 else 0
    TA1_0 = TA1[0] if TA1 else 0
    TBC1 = TBC[1] if len(TBC) > 1 else 0

    seg_rng = [(int(col_base[si]), int(col_base[si + 1]))
               for si in range(len(segs))]
    s0_si = next(i for i, s in enumerate(segs) if s[0] == "s0")
    # chunk regions: pair | direct | singles (same elem size each)
    regions = []
    for si, (kind, j, n) in enumerate(segs):
        knd = ("s" if kind in ("sa1", "sa2", "sb")
               else "p" if kind == "p1" else kind)
        lo, hi = seg_rng[si]
        if regions and regions[-1][2] == knd and regions[-1][1] == lo:
            regions[-1] = (regions[-1][0], hi, knd)
        else:
            regions.append((lo, hi, knd))
    by_kind = {}
    for lo, hi, kind in regions:
        by_kind.setdefault(kind, []).extend(
            (s, min(s + G, hi), kind) for s in range(lo, hi, G))
    # interleave the three independent early streams (quad, pair, direct)
    # to absorb region-transition stalls and smooth the descriptor-size
    # mix on the DMA engines; singles rounds follow
    chunks = []
    streams = [by_kind.get("p4", []), by_kind.get("p", []),
               by_kind.get("s0", [])]
    while any(streams):
        for st in streams:
            if st:
                chunks.append(st.pop(0))
    chunks += by_kind.get("s", [])

    with tile.TileContext(nc) as tc:
        with tc.tile_pool(name="idxp", bufs=1) as idxp, \
             tc.tile_pool(name="accp", bufs=1) as accp, \
             tc.tile_pool(name="gp", bufs=6) as gp, \
             tc.tile_pool(name="pp", bufs=3) as pp, \
             tc.tile_pool(name="qp", bufs=3) as qp:
            idx = idxp.tile([P, C * 8], mybir.dt.int16)
            # dummy 16-idx gather with no data deps: triggers the Q7 mlp
            # library IRAM load during the preamble/idx load
            idxw = idxp.tile([P, 1], mybir.dt.int16)
            nc.gpsimd.memset(idxw[:], 0)
            warm = idxp.tile([P, 1, FEAT], mybir.dt.bfloat16)
            nc.gpsimd.dma_gather(warm[:], t_feat[:], idxw[:], 16, 16, FEAT,
                                 queue_num=0)
            nc.sync.dma_start(out=idx[:], in_=t_idx[:])
            acc = accp.tile([P, TILES, FEAT], mybir.dt.bfloat16)

            pend = []          # pending finalized tile ranges [lo, hi)

            if ztiles > 0:
                # all-zero-degree trailing tiles: no gather needed; zero
                # and write them out while the DMA engines are otherwise
                # idle during the Q7 library load
                nc.scalar.memzero(acc[:, TILES - ztiles:TILES, :])
                nc.sync.dma_start(out=t_oe[:, TILES - ztiles:TILES, :],
                                  in_=acc[:, TILES - ztiles:TILES, :])

            def add_final(lo, hi, force=False):
                if lo < hi:
                    if pend and pend[-1][1] == lo:
                        pend[-1] = (pend[-1][0], hi)
                    elif pend and pend[-1][0] == hi:
                        pend[-1] = (lo, pend[-1][1])
                    else:
                        pend.append((lo, hi))
                keep = []
                for lo, hi in pend:
                    if hi - lo >= WMIN or force:
                        nc.sync.dma_start(out=t_oe[:, lo:hi, :],
                                          in_=acc[:, lo:hi, :])
                    else:
                        keep.append((lo, hi))
                pend[:] = keep

            for k, (cs, ce, kind) in enumerate(chunks):
                w = ce - cs
                qn = (k + 1) % 4
                if kind == "p4":
                    # quad: 4 same-row srcs per 2KB descriptor, A2 tiles
                    g = qp.tile([P, G, 4 * FEAT], mybir.dt.bfloat16,
                                tag="gquad")
                    nc.gpsimd.dma_gather(g[:, :w, :], t_featq[:],
                                         idx[:, cs * 8:ce * 8],
                                         w * P, w * P, 4 * FEAT,
                                         queue_num=qn)
                    # acc = max(g0, g1, g2, g3, 0): init + relu fused
                    nc.vector.scalar_tensor_tensor(
                        out=acc[:, cs:ce, :], in0=g[:, :w, 0:FEAT],
                        scalar=0.0, in1=g[:, :w, FEAT:2 * FEAT],
                        op0=mx, op1=mx)
                    nc.vector.tensor_tensor(
                        out=acc[:, cs:ce, :], in0=acc[:, cs:ce, :],
                        in1=g[:, :w, 2 * FEAT:3 * FEAT], op=mx)
                    nc.vector.tensor_tensor(
                        out=acc[:, cs:ce, :], in0=acc[:, cs:ce, :],
                        in1=g[:, :w, 3 * FEAT:4 * FEAT], op=mx)
                    add_final(max(cs, TA2_0), ce)
                    continue
                if kind == "p":
                    g = pp.tile([P, G, 2 * FEAT], mybir.dt.bfloat16,
                                tag="gpair")
                    nc.gpsimd.dma_gather(g[:, :w, :], t_featp[:],
                                         idx[:, cs * 8:ce * 8],
                                         w * P, w * P, 2 * FEAT,
                                         queue_num=qn)
                    for si, (knd, j, n) in enumerate(segs):
                        if knd != "p1":
                            continue
                        a = max(cs, seg_rng[si][0])
                        b = min(ce, seg_rng[si][1])
                        if a >= b:
                            continue
                        go = a - cs
                        L = b - a
                        tp = nA2 + (a - seg_rng[si][0])
                        # acc = max(max(gA, 0), gB): init + relu fused
                        nc.vector.scalar_tensor_tensor(
                            out=acc[:, tp:tp + L, :],
                            in0=g[:, go:go + L, 0:FEAT], scalar=0.0,
                            in1=g[:, go:go + L, FEAT:2 * FEAT],
                            op0=mx, op1=mx)
                        add_final(max(tp, nA2 + TA1_0), tp + L)
                    continue
                if kind == "s0":
                    lo_t = nA + (cs - seg_rng[s0_si][0])
                    hi_t = nA + (ce - seg_rng[s0_si][0])
                    nc.gpsimd.dma_gather(acc[:, lo_t:hi_t, :], t_feat[:],
                                         idx[:, cs * 8:ce * 8],
                                         w * P, w * P, FEAT, queue_num=qn)
                    # BC tiles with deg<=1: relu on Act, then final
                    lo = max(lo_t, nA + TBC1)
                    if lo < hi_t:
                        nc.scalar.activation(acc[:, lo:hi_t, :],
                                             acc[:, lo:hi_t, :], relu)
                        add_final(lo, hi_t)
                    continue
                g = gp.tile([P, G, FEAT], mybir.dt.bfloat16, tag="g")
                nc.gpsimd.dma_gather(g[:, :w, :], t_feat[:],
                                     idx[:, cs * 8:ce * 8],
                                     w * P, w * P, FEAT, queue_num=qn)
                for si, (knd, j, n) in enumerate(segs):
                    if knd not in ("sa1", "sa2", "sb"):
                        continue
                    a = max(cs, seg_rng[si][0])
                    b = min(ce, seg_rng[si][1])
                    if a >= b:
                        continue
                    go = a - cs
                    L = b - a
                    if knd == "sa2":
                        tp = a - seg_rng[si][0]
                        nxt = TA2[j + 1] if j + 1 < len(TA2) else 0
                        nc.vector.tensor_tensor(
                            out=acc[:, tp:tp + L, :],
                            in0=acc[:, tp:tp + L, :],
                            in1=g[:, go:go + L, :], op=mx)
                        add_final(max(tp, nxt), tp + L)
                    elif knd == "sa1":
                        tp = nA2 + (a - seg_rng[si][0])
                        nxt = TA1[j + 1] if j + 1 < len(TA1) else 0
                        nc.vector.tensor_tensor(
                            out=acc[:, tp:tp + L, :],
                            in0=acc[:, tp:tp + L, :],
                            in1=g[:, go:go + L, :], op=mx)
                        add_final(max(tp, nA2 + nxt), tp + L)
                    else:
                        tp = nA + (a - seg_rng[si][0])
                        nxt = TBC[j + 1] if j + 1 < len(TBC) else 0
                        if j == 1:
                            # first BC reduction: fold the relu in
                            nc.vector.scalar_tensor_tensor(
                                out=acc[:, tp:tp + L, :],
                                in0=acc[:, tp:tp + L, :], scalar=0.0,
                                in1=g[:, go:go + L, :], op0=mx, op1=mx)
                        else:
                            nc.vector.tensor_tensor(
                                out=acc[:, tp:tp + L, :],
                                in0=acc[:, tp:tp + L, :],
                                in1=g[:, go:go + L, :], op=mx)
                        add_final(max(tp, nA + nxt), tp + L)
            add_final(0, 0, force=True)
    nc.compile()
    return nc


def _unshard(results, orders, feat_centers):
    out = np.empty((N_NODES, FEAT), np.float32)
    out[:N_CENTERS] = feat_centers                       # centers: exact copy
    for c in range(NCORES):
        oe = np.asarray(results[c]["out_edge"])          # [P, TILES, FEAT]
        vals = oe.transpose(1, 0, 2).reshape(NPOS, FEAT)  # position-major
        rows = N_CENTERS + c * RC + orders[c]            # position q -> row
        out[rows] = vals[:RC].astype(np.float32)
    return out


def kernel(feat, center_idx, edge_src, edge_dst, n_nodes, _trace=False):
    assert int(n_nodes) == N_NODES
    feat = np.ascontiguousarray(np.asarray(feat, np.float32))
    center_idx = np.asarray(center_idx, np.int64)

    # centers: out[center_idx] = feat, handled fully on the host (pure copy)
    feat_centers = np.zeros((N_CENTERS, FEAT), np.float32)
    feat_centers[center_idx] = feat

    (segs, col_base, C, nA2, nA1, ztiles, TA2, TA1, TBC, tables,
     pair_tables, idx_planes, orders) = _build_plan(edge_src, edge_dst, feat)
    nc = _build_bass(segs, col_base, C, nA2, nA1, ztiles, TA2, TA1, TBC)

    if _trace:
        _install_profile_hook()
    import concourse.bass_utils as bass_utils
    bass_utils.upload_artifacts = lambda tmpdir: f"file://{tmpdir}"
    from concourse.bass_utils import run_bass_kernel_spmd

    in_maps = [{"feat_tbl": tables[c], "feat_tblp": pair_tables[c],
                "feat_tblq": tables[c].reshape(TBL // 4, 4 * FEAT),
                "idxs": idx_planes[c]} for c in range(NCORES)]
    kw = dict(trace=True) if _trace else {}
    res = run_bass_kernel_spmd(nc, in_maps, list(range(NCORES)), **kw)

    out = _unshard(res.results, orders, feat_centers)
    if _trace:
        return out, res
    return out
